# revision 1
# baseline (speedup 1.0000x reference)
"""GNN message-passing kernel (kapitza edge energies + segment_sum) on 8 TRN2 cores.

Strategy
--------
Shard by *target-node range*: core k owns nodes [k*125000, (k+1)*125000) and
receives every edge whose tgt falls in that range (host sorts edges by tgt).
Each core's output slice is independent -> no collective; host concatenates.

On each core the host lays edges out in a degree-padded CSR grid
[rows x PAD1] (row == local target node, PAD1=16 slots each).  That makes
  * T[tgt] a dense per-row broadcast (no gather),
  * the segment-sum a dense free-axis reduction (no scatter),
and leaves ONE indexed op: gathering the u64-packed {T, W}[src] pair
(8 B per edge) from an on-device node table.  W = 0.5*pi*L*D*avg_f is
precomputed densely on device.

The gather uses gpsimd indirect DMA with the empirically-validated HW
contract: a [1, K, 1] destination AP based at partition p yields K
per-element descriptors; descriptor j consumes index idx[j % 128, j // 128]
(column-major across the index tile's partitions); indices > bounds_check
are skipped leaving the destination untouched (pad slots).  One instruction
per partition (128 per core) gathers all 16K slot pairs of that partition.

Nodes with degree > PAD1 overflow (<= 504 per core for this data) into a
small second grid processed on 16 partitions at the end, whose row sums are
added into the output via small indirect gather/add/scatter.
"""

import sys
from contextlib import ExitStack
from dataclasses import dataclass

import numpy as np

if "/opt/trn_rl_repo" not in sys.path:
    sys.path.insert(0, "/opt/trn_rl_repo")

import concourse.bacc as bacc
import concourse.bass as bass
import concourse.tile as tile
from concourse import mybir
import concourse.mybir as mb
from concourse.tile_rust import add_dep_helper

F32 = mybir.dt.float32
I32 = mybir.dt.int32
U64 = mybir.dt.uint64
PI = float(np.pi)


@dataclass(frozen=True)
class Cfg:
    N_NODES: int  # real node count
    NT_PART: int  # node-table cols per partition (NT_PAD = 128*NT_PART)
    NT_CHUNKS: int  # phase-A chunks
    NODES_PC: int  # real nodes per core
    R_PART: int  # padded rows (local nodes) per partition
    N_CHUNK: int  # compute chunks; CHUNK_R = R_PART // N_CHUNK
    PAD1: int  # level-1 slots per row; SLOTS_P = R_PART*PAD1 must be %128==0
    OV_ROWS: int  # overflow rows (spread over OV_PARTS partitions)
    OV_PAD: int  # overflow slots per row
    OV_PARTS: int = 16
    tiers: tuple = ()  # ((pad, rows_per_partition), ...) two-tier row padding
    n_queues: int = 1  # SWDGE queues for gather instructions
    table_sbuf: bool = False  # node table resident in SBUF (encoded indices)
    sim_order: bool = False  # True: interp's row-major idx order (CoreSim)
    loop_reps: int = 0  # timing variant: repeat gather+compute in a For_i
    debug: bool = False

    @property
    def NT_PAD(self):
        return 128 * self.NT_PART

    @property
    def NT_CR(self):
        return self.NT_PART // self.NT_CHUNKS

    @property
    def ROWS(self):
        return 128 * self.R_PART

    @property
    def CHUNK_R(self):
        return self.R_PART // self.N_CHUNK

    @property
    def CHUNK_S(self):
        return self.CHUNK_R * self.PAD1

    @property
    def SLOTS_P(self):
        if self.tiers:
            return sum(pad * rpp for pad, rpp in self.tiers)
        return self.R_PART * self.PAD1  # slots per partition

    @property
    def R_PART_EFF(self):
        return sum(rpp for _, rpp in self.tiers) if self.tiers else self.R_PART

    @property
    def IDX_C(self):
        return self.SLOTS_P // 128  # idx-tile columns per partition-instruction

    @property
    def OV_S(self):
        return self.OV_ROWS * self.OV_PAD  # total overflow slots

    @property
    def OV_RPP(self):
        return self.OV_ROWS // self.OV_PARTS  # overflow rows per used partition

    @property
    def OV_SPP(self):
        return self.OV_S // self.OV_PARTS  # overflow slots per used partition


FULL = Cfg(
    N_NODES=1_000_000,
    NT_PART=7816,  # NT_PAD = 1_000_448
    NT_CHUNKS=8,
    NODES_PC=125_000,
    R_PART=1024,  # ROWS = 131_072; SLOTS_P = 16384 = 128*128
    N_CHUNK=8,
    PAD1=16,
    OV_ROWS=1024,  # measured: <= 504 used per core
    OV_PAD=16,  # measured max degree 25 -> needs >= 9 (over pad-16 tier)
    table_sbuf=True,
    tiers=((8, 592), (16, 408)),  # SLOTS_P = 11264 vs 16384 uniform
)

N_CORES = 8


PSTRIDE64 = 256 * 1024 // 8  # SBUF partition stride in u64 elements (cayman)


def _sentinel(cfg):
    if cfg.table_sbuf:
        return np.int32(1 << 23)
    return np.int32(cfg.N_NODES)  # > bounds_check = N_NODES-1 -> skipped


def _encode_idx(cfg, n):
    """Node id -> gather index (SBUF mode encodes the partition stride)."""
    if not cfg.table_sbuf:
        return n.astype(np.int32)
    return ((n // cfg.NT_PART) * PSTRIDE64 + (n % cfg.NT_PART)).astype(np.int32)


# --------------------------------------------------------------------------
# device program
# --------------------------------------------------------------------------
def build_program(cfg: Cfg):
    assert cfg.SLOTS_P % 128 == 0 and cfg.OV_S % 128 == 0 and cfg.OV_ROWS % 128 == 0
    nc = bacc.Bacc(
        "TRN2", target_bir_lowering=False, debug=cfg.debug,
        num_swdge_queues=max(cfg.n_queues, 1),
    )

    Tf = nc.dram_tensor("T_full", [128, cfg.NT_PART], F32, kind="ExternalInput")
    Lf = nc.dram_tensor("L_full", [128, cfg.NT_PART], F32, kind="ExternalInput")
    Df = nc.dram_tensor("D_full", [128, cfg.NT_PART], F32, kind="ExternalInput")
    Ff = nc.dram_tensor("F_full", [128, cfg.NT_PART], F32, kind="ExternalInput")
    Tloc = nc.dram_tensor("T_loc", [128, cfg.R_PART_EFF], F32, kind="ExternalInput")
    ts = nc.dram_tensor("ts", [128, 1], F32, kind="ExternalInput")
    cond1 = nc.dram_tensor("cond1", [128, cfg.SLOTS_P], F32, kind="ExternalInput")
    # per-partition-instruction index tiles: cols [p*IDX_C, (p+1)*IDX_C)
    src1t = nc.dram_tensor("src1t", [128, cfg.SLOTS_P], I32, kind="ExternalInput")
    ovc = nc.dram_tensor("ov_cond", [cfg.OV_PARTS, cfg.OV_SPP], F32, kind="ExternalInput")
    ovst = nc.dram_tensor("ov_srct", [128, cfg.OV_S // 128], I32, kind="ExternalInput")
    ovnt = nc.dram_tensor("ov_nodet", [128, cfg.OV_ROWS // 128], I32, kind="ExternalInput")
    ovot = nc.dram_tensor("ov_outt", [128, cfg.OV_ROWS // 128], I32, kind="ExternalInput")
    out = nc.dram_tensor("out", [128, cfg.R_PART_EFF], F32, kind="ExternalOutput")

    # u64-packed {T, W} node table, built on device
    table = nc.dram_tensor("tableTW", [cfg.NT_PAD, 2], F32)
    table_w = table[:].rearrange("(p f) two -> p (f two)", p=128)
    table64 = table[:].bitcast(U64)  # [NT_PAD, 1]

    def q_of(i):
        qi = i % max(cfg.n_queues, 1)
        return f"qPoolDynamic{qi or ''}"

    sbuf_src = {"ap": None}  # set to the SBUF table AP in table_sbuf mode

    def gather64(out_ap_1k1, idx_ap, deps, qname):
        if not cfg.table_sbuf:
            g = nc.gpsimd.indirect_dma_start(
                out=out_ap_1k1,
                out_offset=None,
                in_=table64,
                in_offset=bass.IndirectOffsetOnAxis(ap=idx_ap, axis=0),
                bounds_check=cfg.N_NODES - 1,
                oob_is_err=False,
            )
            g.ins.queue = qname
        else:
            gp = nc.gpsimd
            out_l = gp.lower_ap_dma(out_ap_1k1, for_indirect_dma=True)
            in_l = gp.lower_ap_dma(sbuf_src["ap"], for_indirect_dma=True)
            off_l = gp.lower_ap_dma(idx_ap)
            assert len(out_l) == 1 and len(in_l) == 1 and len(off_l) == 1
            in_l.append(off_l[0])
            in_l[0].dynamic_ap_info = mb.DynamicAccessPatternInfo(
                c=0,
                actual_ap=out_l[0].ap,
                indirect_dim_max_index=cfg.NT_PAD,
                offset_expr=[
                    mb.DynamicAccessPatternOffsetExpr(
                        coef=1,
                        aff_expr=mb.DynamicAccessPatternOffsetExprAffExpr(
                            kind="IndirectArgId", arg_id=1
                        ),
                    )
                ],
            )
            bound = 127 * PSTRIDE64 + cfg.NT_PART
            bc = [gp.lower_val_access(gp.to_reg(int(bound)))]
            g = gp.add_instruction(
                mb.InstDMACopy(
                    name=nc.get_next_instruction_name(),
                    queue=qname,
                    mode="Copy",
                    ins=in_l + bc,
                    outs=out_l,
                    oob_is_err=False,
                    cce_op=mb.AluOpType.bypass,
                )
            )
        for w in deps:
            add_dep_helper(g.ins, w.ins, reason="table RAW")
        return g

    with tile.TileContext(nc) as tc, ExitStack() as ctx:
        pts = ctx.enter_context(tc.tile_pool(name="pts", bufs=1))
        tts = pts.tile([128, 1], F32)
        nc.sync.dma_start(tts[:], ts[:])
        ttab = None
        if cfg.table_sbuf:
            ttab = pts.tile([128, 2 * cfg.NT_PART], F32)
            sbuf_src["ap"] = ttab[0:1, :].bitcast(U64)

        # ---- phase A: W = 0.5*pi*L*D*avg_f; write packed {T,W} table ----
        pa_ctx = ExitStack()
        pa = pa_ctx.enter_context(tc.tile_pool(name="pa", bufs=2))
        table_writes = []
        for j in range(cfg.NT_CHUNKS):
            sl = slice(j * cfg.NT_CR, (j + 1) * cfg.NT_CR)
            tT = pa.tile([128, cfg.NT_CR], F32, tag="tT")
            nc.sync.dma_start(tT[:], Tf[:, sl])
            tL = pa.tile([128, cfg.NT_CR], F32, tag="tL")
            nc.sync.dma_start(tL[:], Lf[:, sl])
            tD = pa.tile([128, cfg.NT_CR], F32, tag="tD")
            nc.sync.dma_start(tD[:], Df[:, sl])
            tF = pa.tile([128, cfg.NT_CR], F32, tag="tF")
            nc.sync.dma_start(tF[:], Ff[:, sl])
            tW = pa.tile([128, cfg.NT_CR], F32, tag="tW")
            nc.vector.tensor_mul(tW[:], tL[:], tD[:])
            nc.vector.tensor_mul(tW[:], tW[:], tF[:])
            nc.vector.tensor_scalar_mul(tW[:], tW[:], 0.5 * PI)
            if cfg.table_sbuf:
                dst = ttab[:, j * cfg.NT_CR * 2 : (j + 1) * cfg.NT_CR * 2]
                ev = dst.rearrange("p (f two) -> p f two", two=2)
                w1 = nc.vector.tensor_copy(ev[:, :, 0:1], tT[:].unsqueeze(2))
                w2 = nc.vector.tensor_copy(ev[:, :, 1:2], tW[:].unsqueeze(2))
                table_writes.extend([w1, w2])
            else:
                tTW = pa.tile([128, cfg.NT_CR * 2], F32, tag="tTW")
                ev = tTW[:].rearrange("p (f two) -> p f two", two=2)
                nc.vector.tensor_copy(ev[:, :, 0:1], tT[:].unsqueeze(2))
                nc.vector.tensor_copy(ev[:, :, 1:2], tW[:].unsqueeze(2))
                w = nc.sync.dma_start(
                    table_w[:, j * cfg.NT_CR * 2 : (j + 1) * cfg.NT_CR * 2], tTW[:]
                )
                table_writes.append(w)
        pa_ctx.close()

        # ---- phase B: local T, T^3 * time_step ----
        main_ctx = ExitStack()
        pers = main_ctx.enter_context(tc.tile_pool(name="pers", bufs=1))
        tTloc = pers.tile([128, cfg.R_PART_EFF], F32)
        nc.sync.dma_start(tTloc[:], Tloc[:])
        tT3 = pers.tile([128, cfg.R_PART_EFF], F32)
        nc.vector.tensor_mul(tT3[:], tTloc[:], tTloc[:])
        nc.vector.tensor_mul(tT3[:], tT3[:], tTloc[:])
        nc.vector.tensor_scalar(
            tT3[:], tT3[:], tts[:, 0:1], None, op0=mybir.AluOpType.mult
        )

        # ---- gather + compute, in half-sweeps when the SBUF table is resident ----
        n_halves = 2 if cfg.table_sbuf else 1
        assert cfg.SLOTS_P % (128 * n_halves) == 0
        assert cfg.N_CHUNK % n_halves == 0
        HS = cfg.SLOTS_P // n_halves  # slots per half per partition
        ptw = main_ctx.enter_context(tc.tile_pool(name="ptw", bufs=1))
        pidx = main_ctx.enter_context(tc.tile_pool(name="pidx", bufs=4))
        pc = main_ctx.enter_context(tc.tile_pool(name="pc", bufs=2))
        loop_ctx = ExitStack()
        if cfg.loop_reps > 0:
            loop_ctx.enter_context(tc.For_i(0, cfg.loop_reps, 1))
        # chunk plan: (pad, row0, nrows, slot0), each within one half
        if cfg.tiers:
            regions = []
            ro = so = 0
            for pad, rpp in cfg.tiers:
                regions.append((pad, rpp, ro, so))
                ro += rpp
                so += pad * rpp
        else:
            regions = [(cfg.PAD1, cfg.R_PART, 0, 0)]
        chunk_plan = []  # per half: list of chunks
        for h in range(n_halves):
            lo, hi = h * HS, (h + 1) * HS
            cl = []
            for pad, rpp, ro, so in regions:
                a, b = max(so, lo), min(so + pad * rpp, hi)
                s = a
                while s < b:
                    e = min(s + 2048, b)
                    e = s + ((e - s) // pad) * pad
                    cl.append((pad, ro + (s - so) // pad, (e - s) // pad, s))
                    s = e
            chunk_plan.append(cl)
        out_writes = []
        for h in range(n_halves):
            h0 = h * HS
            TWh = ptw.tile([128, 2 * HS], F32, tag="tw")
            nc.vector.memset(TWh[:], 0.0)
            cols0 = h0 // 128
            ncols = HS // 128
            for p in range(128):
                ti = pidx.tile([128, ncols], I32, tag="idx")
                nc.sync.dma_start(
                    ti[:],
                    src1t[:, p * cfg.IDX_C + cols0 : p * cfg.IDX_C + cols0 + ncols],
                )
                gather64(
                    TWh[p : p + 1, :].bitcast(U64).unsqueeze(2),
                    ti[:],
                    table_writes,
                    q_of(p),
                )

            for pad, r0, nr, s0 in chunk_plan[h]:
                cs = nr * pad
                ls = s0 - h0  # chunk offset within this half
                tcond = pc.tile([128, 2048], F32, tag="cond")
                nc.sync.dma_start(tcond[:, :cs], cond1[:, s0 : s0 + cs])
                tw4 = TWh[:, 2 * ls : 2 * (ls + cs)].rearrange(
                    "p (r s two) -> p r s two", s=pad, two=2
                )
                Ts = tw4[:, :, :, 0:1].squeeze(3)
                Ws = tw4[:, :, :, 1:2].squeeze(3)
                Tt = tTloc[:, r0 : r0 + nr].unsqueeze(2).broadcast_to([128, nr, pad])
                T3 = tT3[:, r0 : r0 + nr].unsqueeze(2).broadcast_to([128, nr, pad])
                te = pc.tile([128, 2048], F32, tag="e")
                te3 = te[:, :cs].rearrange("p (r s) -> p r s", s=pad)
                nc.vector.tensor_tensor(te3, Ts, Tt, op=mybir.AluOpType.subtract)
                nc.vector.tensor_scalar_max(te[:, :cs], te[:, :cs], 0.0)
                nc.vector.tensor_tensor(te3, te3, Ws, op=mybir.AluOpType.mult)
                nc.vector.tensor_mul(te[:, :cs], te[:, :cs], tcond[:, :cs])
                nc.vector.tensor_tensor(te3, te3, T3, op=mybir.AluOpType.mult)
                tred = pc.tile([128, 512], F32, tag="red")
                nc.vector.tensor_reduce(
                    tred[:, :nr], te3, axis=mybir.AxisListType.X, op=mybir.AluOpType.add
                )
                w = nc.sync.dma_start(out[:, r0 : r0 + nr], tred[:, :nr])
                out_writes.append(w)

        loop_ctx.close()
        main_ctx.close()
        if cfg.loop_reps == 0:
            # ---- phase D: overflow rows (degree > PAD1) on OV_PARTS partitions ----
            po = ctx.enter_context(tc.tile_pool(name="po", bufs=1))
            pidx = ctx.enter_context(tc.tile_pool(name="pidx2", bufs=2))
            NP, SPP, RPP = cfg.OV_PARTS, cfg.OV_SPP, cfg.OV_RPP
            toc = po.tile([128, SPP], F32)
            nc.sync.dma_start(toc[:NP, :], ovc[:])
            # slot pairs: NP instructions, one per used partition
            toTW = po.tile([128, 2 * SPP], F32)
            nc.vector.memset(toTW[:], 0.0)
            ovs_cols = cfg.OV_S // 128 // NP  # idx cols per partition-instruction
            for q in range(NP):
                ti = pidx.tile([128, ovs_cols], I32, tag="ovidx")
                nc.sync.dma_start(ti[:], ovst[:, q * ovs_cols : (q + 1) * ovs_cols])
                gather64(
                    toTW[q : q + 1, :].bitcast(U64).unsqueeze(2), ti[:], table_writes,
                    q_of(q),
                )
            # target-T pairs for all OV_ROWS via one instruction into partition 0
            tno = pidx.tile([128, cfg.OV_ROWS // 128], I32, tag="ovn")
            nc.sync.dma_start(tno[:], ovnt[:])
            toTt0 = po.tile([128, 2 * cfg.OV_ROWS], F32)  # partition 0 row used
            nc.vector.memset(toTt0[0:1, :], 0.0)
            gather64(
                toTt0[0:1, :].bitcast(U64).unsqueeze(2), tno[:], table_writes, q_of(0)
            )
            # spread partition-0 pair stream -> [NP, 2*RPP]
            toTt = po.tile([128, 2 * RPP], F32)
            nc.sync.dma_start(toTt[:NP, :], toTt0[0:1, :])

            ttpair = toTt[:NP, :].rearrange("p (r two) -> p r two", two=2)
            tt_col = ttpair[:, :, 0:1]  # [NP, RPP, 1] target T
            to3 = po.tile([128, RPP], F32)
            nc.vector.tensor_tensor(
                to3[:NP, :].unsqueeze(2), tt_col, tt_col, op=mybir.AluOpType.mult
            )
            nc.vector.tensor_tensor(
                to3[:NP, :].unsqueeze(2), to3[:NP, :].unsqueeze(2), tt_col,
                op=mybir.AluOpType.mult,
            )
            nc.vector.tensor_scalar(
                to3[:NP, :], to3[:NP, :], tts[:NP, 0:1], None, op0=mybir.AluOpType.mult
            )

            ow4 = toTW[:NP, :].rearrange("p (r s two) -> p r s two", s=cfg.OV_PAD, two=2)
            oTs = ow4[:, :, :, 0:1].squeeze(3)
            oWs = ow4[:, :, :, 1:2].squeeze(3)
            oTt = tt_col.broadcast_to([NP, RPP, cfg.OV_PAD])
            oT3 = to3[:NP, :].unsqueeze(2).broadcast_to([NP, RPP, cfg.OV_PAD])
            teo = po.tile([128, SPP], F32)
            teo3 = teo[:NP, :].rearrange("p (r s) -> p r s", s=cfg.OV_PAD)
            nc.vector.tensor_tensor(teo3, oTs, oTt, op=mybir.AluOpType.subtract)
            nc.vector.tensor_scalar_max(teo[:NP, :], teo[:NP, :], 0.0)
            nc.vector.tensor_tensor(teo3, teo3, oWs, op=mybir.AluOpType.mult)
            nc.vector.tensor_mul(teo[:NP, :], teo[:NP, :], toc[:NP, :])
            nc.vector.tensor_tensor(teo3, teo3, oT3, op=mybir.AluOpType.mult)
            tosum = po.tile([128, RPP], F32)
            nc.vector.tensor_reduce(
                tosum[:NP, :], teo3, axis=mybir.AxisListType.X, op=mybir.AluOpType.add
            )
            # collapse [NP, RPP] -> partition-0 stream [1, OV_ROWS]
            tosum0 = po.tile([128, cfg.OV_ROWS], F32)
            nc.sync.dma_start(tosum0[0:1, :], tosum[:NP, :])

            # gather current out rows, add, scatter back (f32 elements)
            too = pidx.tile([128, cfg.OV_ROWS // 128], I32, tag="ovo")
            nc.sync.dma_start(too[:], ovot[:])
            out_flat = out[:].rearrange("p r -> (p r)").unsqueeze(1)
            tcur = po.tile([128, cfg.OV_ROWS], F32)
            g3 = nc.gpsimd.indirect_dma_start(
                out=tcur[0:1, :].unsqueeze(2),
                out_offset=None,
                in_=out_flat,
                in_offset=bass.IndirectOffsetOnAxis(ap=too[:], axis=0),
                bounds_check=None,
                oob_is_err=False,
            )
            for w in out_writes:
                add_dep_helper(g3.ins, w.ins, reason="out RAW before ov add")
            nc.vector.tensor_add(tosum0[0:1, :], tosum0[0:1, :], tcur[0:1, :])
            sc = nc.gpsimd.indirect_dma_start(
                out=out_flat,
                out_offset=bass.IndirectOffsetOnAxis(ap=too[:], axis=0),
                in_=tosum0[0:1, :].unsqueeze(2),
                in_offset=None,
                bounds_check=None,
                oob_is_err=False,
            )
            for w in out_writes:
                add_dep_helper(sc.ins, w.ins, reason="out WAW after level-1")
            add_dep_helper(sc.ins, g3.ins, reason="ov scatter after gather")

    nc.compile()
    return nc


# --------------------------------------------------------------------------
# host-side sharding / layout
# --------------------------------------------------------------------------
def _wrap_cols(cfg: Cfg, flat, cols):
    """Lay a flat stream so HW consumes it in order: idx[j%128, j//128]=flat[j].

    With sim_order (interp semantics) consumption is row-major over [128, C]."""
    if cfg.sim_order:
        return flat.reshape(128, cols)
    return np.ascontiguousarray(flat.reshape(cols, 128).T)


_ROWMAPS = {}


def host_prep(cfg: Cfg, T, L, D, avg_f, conductivity, src, tgt, time_step):
    T = np.asarray(T, np.float32)
    L = np.asarray(L, np.float32)
    D = np.asarray(D, np.float32)
    avg_f = np.asarray(avg_f, np.float32)
    cond = np.asarray(conductivity, np.float32)
    src = np.asarray(src, np.int32)
    tgt = np.asarray(tgt, np.int32)
    ts = np.full((128, 1), np.float32(np.asarray(time_step)), np.float32)

    def padded(x):
        p = np.zeros(cfg.NT_PAD, np.float32)
        p[: cfg.N_NODES] = x
        return p.reshape(128, cfg.NT_PART)

    Tp, Lp, Dp, Fp = padded(T), padded(L), padded(D), padded(avg_f)

    order = np.argsort(tgt, kind="stable")
    tgt_s = tgt[order]
    src_s = src[order]
    cond_s = cond[order]
    SENT = _sentinel(cfg)

    in_maps = []
    for k in range(N_CORES):
        base = k * cfg.NODES_PC
        lo, hi = np.searchsorted(tgt_s, [base, base + cfg.NODES_PC])
        n = (tgt_s[lo:hi] - base).astype(np.int64)
        s = src_s[lo:hi]
        c = cond_s[lo:hi]

        deg = np.bincount(n, minlength=cfg.ROWS)
        starts = np.concatenate([[0], np.cumsum(deg)[:-1]])
        rank = np.arange(len(n), dtype=np.int64) - starts[n]

        REFF = cfg.R_PART_EFF
        if cfg.tiers:
            (p8, rpp8), (p16, rpp16) = cfg.tiers
            nodes = np.arange(cfg.NODES_PC, dtype=np.int64)
            hi = deg[: cfg.NODES_PC] > p8
            lo_n = nodes[~hi]
            hi_n = nodes[hi]
            assert len(lo_n) <= rpp8 * 128, (len(lo_n), rpp8 * 128)
            assert len(hi_n) <= (rpp16 - 1) * 128, (len(hi_n), rpp16 * 128)
            rowp = np.empty(cfg.NODES_PC, np.int64)
            rowr = np.empty(cfg.NODES_PC, np.int64)
            i8 = np.arange(len(lo_n))
            rowp[lo_n] = i8 % 128
            rowr[lo_n] = i8 // 128
            i16 = np.arange(len(hi_n))
            rowp[hi_n] = i16 % 128
            rowr[hi_n] = rpp8 + i16 // 128
            padv = np.where(hi, p16, p8)
            sbase = np.where(
                hi, rpp8 * p8 + (rowr - rpp8) * p16, rowr * p8
            )
            pad1_eff = padv[n]
            m1 = rank < pad1_eff
            cond1 = np.zeros(128 * cfg.SLOTS_P, np.float32)
            src1 = np.full(128 * cfg.SLOTS_P, SENT, np.int32)
            slot = rowp[n] * cfg.SLOTS_P + sbase[n] + rank
            cond1[slot[m1]] = c[m1]
            src1[slot[m1]] = _encode_idx(cfg, s[m1].astype(np.int64))
            rowmap = rowp * REFF + rowr  # node -> out flat index
            _ROWMAPS[k] = rowmap
        else:
            m1 = rank < cfg.PAD1
            cond1 = np.zeros(cfg.ROWS * cfg.PAD1, np.float32)
            src1 = np.full(cfg.ROWS * cfg.PAD1, SENT, np.int32)
            slot = n[m1] * cfg.PAD1 + rank[m1]
            cond1[slot] = c[m1]
            src1[slot] = _encode_idx(cfg, s[m1].astype(np.int64))
        # per-partition idx tiles, column-major wrapped
        src1_p = src1.reshape(128, cfg.SLOTS_P)
        src1t = np.concatenate(
            [_wrap_cols(cfg, src1_p[p], cfg.IDX_C) for p in range(128)], axis=1
        )

        m2 = ~m1
        ov_nodes = np.unique(n[m2])
        assert len(ov_nodes) <= cfg.OV_ROWS, (len(ov_nodes), cfg.OV_ROWS)
        maxpad = cfg.tiers[-1][0] if cfg.tiers else cfg.PAD1
        assert deg.max() <= maxpad + cfg.OV_PAD, deg.max()
        ov_row_of = np.full(cfg.ROWS, -1, np.int64)
        ov_row_of[ov_nodes] = np.arange(len(ov_nodes))
        ov_cond = np.zeros(cfg.OV_S, np.float32)
        ov_src = np.full(cfg.OV_S, SENT, np.int32)
        ovslot = ov_row_of[n[m2]] * cfg.OV_PAD + (rank[m2] - cfg.PAD1)
        ov_cond[ovslot] = c[m2]
        ov_src[ovslot] = _encode_idx(cfg, s[m2].astype(np.int64))
        ov_node_g = np.full(cfg.OV_ROWS, SENT, np.int32)
        ov_node_g[: len(ov_nodes)] = _encode_idx(cfg, (base + ov_nodes).astype(np.int64))
        ov_out = np.full(cfg.OV_ROWS, 128 * REFF - 1, np.int32)
        if cfg.tiers:
            ov_out[: len(ov_nodes)] = rowmap[ov_nodes].astype(np.int32)
        else:
            ov_out[: len(ov_nodes)] = ov_nodes.astype(np.int32)
        # overflow slot pairs: OV_PARTS per-partition instructions
        ovsp = ov_src.reshape(cfg.OV_PARTS, cfg.OV_SPP)
        ovs_cols = cfg.OV_S // 128 // cfg.OV_PARTS
        ov_srct = np.concatenate(
            [_wrap_cols(cfg, ovsp[q], ovs_cols) for q in range(cfg.OV_PARTS)], axis=1
        )
        ov_nodet = _wrap_cols(cfg, ov_node_g, cfg.OV_ROWS // 128)
        ov_outt = _wrap_cols(cfg, ov_out, cfg.OV_ROWS // 128)

        Tl = np.zeros(128 * REFF, np.float32)
        if cfg.tiers:
            Tl[rowmap] = T[base : base + cfg.NODES_PC]
        else:
            Tl[: cfg.NODES_PC] = T[base : base + cfg.NODES_PC]

        in_maps.append(
            {
                "T_full": Tp,
                "L_full": Lp,
                "D_full": Dp,
                "F_full": Fp,
                "T_loc": Tl.reshape(128, REFF),
                "ts": ts,
                "cond1": cond1.reshape(128, cfg.SLOTS_P),
                "src1t": src1t,
                "ov_cond": ov_cond.reshape(cfg.OV_PARTS, cfg.OV_SPP),
                "ov_srct": ov_srct,
                "ov_nodet": ov_nodet,
                "ov_outt": ov_outt,
            }
        )
    return in_maps


def unshard(cfg: Cfg, results):
    outs = []
    for k in range(N_CORES):
        o = np.asarray(results[k]["out"], np.float32).reshape(128 * cfg.R_PART_EFF)
        if cfg.tiers:
            outs.append(o[_ROWMAPS[k]])
        else:
            outs.append(o[: cfg.NODES_PC])
    return np.concatenate(outs)


# --------------------------------------------------------------------------
# entry point
# --------------------------------------------------------------------------
_NC_CACHE = {}


def _get_program(cfg: Cfg):
    if cfg not in _NC_CACHE:
        _NC_CACHE[cfg] = build_program(cfg)
    return _NC_CACHE[cfg]


def kernel(**inputs) -> np.ndarray:
    from concourse.bass_utils import run_bass_kernel_spmd

    cfg = FULL
    nc = _get_program(cfg)
    in_maps = host_prep(cfg, **inputs)
    res = run_bass_kernel_spmd(nc, in_maps, core_ids=list(range(N_CORES)))
    return unshard(cfg, res.results)

